# revision 34
# baseline (speedup 1.0000x reference)
"""Tensor-parallel DeepSpeed encoder-decoder block on 8 TRN2 NeuronCores.

Sharding (mp_group scheme): attn_qkvw / inter_w / inter_w1 column-sharded
(2 heads / 512 intermediate cols per core), attn_ow / output_w row-sharded.
Post-attn all-reduce = ReduceScatter + AllGather (fp8 payloads, x64 scale
folded host-side); post-output_w all-reduce = ReduceScatter only (fp8, x256
scale), each core finishing its own 128-row feature stripe.

Device compute: fp8(e4m3) DoubleRow matmuls (K=256 per instruction) for the
QKV / MLP / probs-V GEMMs; bf16 for scores and attn-out. LayerNorms fold
gamma/beta into weights host-side; mean correction is a rank-1 matmul; the
1/std factor is Sqrt on Scalar + reciprocal on DVE (rows reshaped to
[128,N] via SBUF->SBUF DMA so the iterative divide runs 128 lanes wide),
broadcast via a K=1 matmul and applied at PSUM drain.  All Sqrt calls are
batched so the Scalar engine's activation-table set switches only a handful
of times across the whole kernel (Exp for attention, Sqrt for the two LN
stats batches, Gelu for the MLP).  LN2 stats ride inside the AllGather as 2
extra rows per rank (per-stripe sum / sum-of-squares of the residual), so
no extra collective and no second pass over the activations.  Softmax runs
in transposed layout with the denominator produced by a ones-column
augmentation of V; its reciprocal also takes the DVE [128,N] path.  GpSimd
handles collective triggers plus pre-collective elementwise work only, so
compute queues never block on collective waits.
"""
from contextlib import ExitStack

import numpy as np
import ml_dtypes

import concourse.bacc as bacc
import concourse.mybir as mybir
import concourse.tile as tile
from concourse import masks
from concourse.bass_utils import run_bass_kernel_spmd

f32 = mybir.dt.float32
bf16 = mybir.dt.bfloat16
fp8 = mybir.dt.float8e4
AF = mybir.ActivationFunctionType
ALU = mybir.AluOpType
DR = mybir.MatmulPerfMode.DoubleRow

NC = 8
B, S, D, I = 2, 2048, 1024, 4096
H, HD = 16, 64
T = B * S
DC = D // 128          # 8 feature chunks
NQKV = 384             # qkv cols per core
EPS = 1e-12

SIM_GELU = False       # sim doesn't implement Gelu; swap for Sigmoid there

SW = 16.0              # fp8 weight scale (wqkv, w1, w2, outw)
SCTX = 32.0            # ctxT scale
SAR = 64.0             # attn partial / RS / AG scale
SO2 = 256.0            # mlp partial / RS2 scale

_BF = ml_dtypes.bfloat16
_F8 = ml_dtypes.float8_e4m3fn


def _bf(a):
    return np.ascontiguousarray(np.asarray(a, np.float32).astype(_BF))


def _f8(a):
    return np.ascontiguousarray(
        np.clip(np.asarray(a, np.float32), -240.0, 240.0).astype(_F8))


def _planes(w):  # [D, F] -> [128, D//128, F]
    w = np.asarray(w, np.float32)
    return w.reshape(DC, 128, w.shape[1]).transpose(1, 0, 2)


def _build():
    nc = bacc.Bacc("TRN2", target_bir_lowering=False, debug=False,
                   num_devices=NC)

    inp = {}

    def din(name, shape, dt):
        inp[name] = nc.dram_tensor(name, shape, dt, kind="ExternalInput")
        return inp[name]

    xp = din("xp", [128, DC, T], fp8)          # x feature-planes
    x_own = din("x_own", [128, T], f32)        # core's 128-feat stripe of x
    wqkv = din("wqkv", [128, DC, NQKV], fp8)   # 16x folded-LN1 qkv weights
    ncsq = din("ncsq", [1, NQKV], f32)         # -colsum(wqkv_dev)/D
    ow = din("ow", [128, D], bf16)             # 2x attn_ow rows
    w1 = din("w1", [128, DC, 512], fp8)        # 16x folded-LN2 inter_w
    ncs1 = din("ncs1", [1, 512], f32)          # -colsum(w1_dev)/SW
    w2 = din("w2", [128, DC, 512], fp8)        # inter_w1 / 4
    outw = din("outw", [128, 4, D], fp8)       # 8x output_w
    seli = din("seli", [16, 2], fp8)           # LN2 stats combiner

    outT = nc.dram_tensor("outT", [128, T], f32, kind="ExternalOutput")

    RG = [list(range(NC))]

    with tile.TileContext(nc) as tc:
        with ExitStack() as ctx:
            ep = ctx.enter_context
            cons = ep(tc.tile_pool(name="cons", bufs=1))
            wp = ep(tc.tile_pool(name="wp", bufs=1))
            qkvp = ep(tc.tile_pool(name="qkvp", bufs=1))
            rowp = ep(tc.tile_pool(name="rowp", bufs=3))
            sqp = ep(tc.tile_pool(name="sqp", bufs=3))
            xbp = ep(tc.tile_pool(name="xbp", bufs=3))
            vaugp = ep(tc.tile_pool(name="vaugp", bufs=9))
            expp = ep(tc.tile_pool(name="expp", bufs=4))
            drp = ep(tc.tile_pool(name="drp", bufs=4))
            agp = ep(tc.tile_pool(name="agp", bufs=5))
            rsdp = ep(tc.tile_pool(name="rsdp", bufs=5))
            gip = ep(tc.tile_pool(name="gip", bufs=2))
            h1p = ep(tc.tile_pool(name="h1p", bufs=2))
            itp = ep(tc.tile_pool(name="itp", bufs=4))
            wfp = ep(tc.tile_pool(name="wfp", bufs=5))
            # PSUM banks: psA 2 + psB 1 + psS 4 + psT 1 = 8
            psA = ep(tc.tile_pool(name="psA", bufs=2, space="PSUM"))
            psB = ep(tc.tile_pool(name="psB", bufs=1, space="PSUM"))
            psS = ep(tc.tile_pool(name="psS", bufs=1, space="PSUM"))
            psT = ep(tc.tile_pool(name="psT", bufs=1, space="PSUM"))
            dram = ep(tc.tile_pool(name="dram", bufs=1, space="DRAM"))

            # ---------------- constants ----------------
            ident = cons.tile([128, 64], bf16)
            masks.make_identity(nc, ident[0:64, :])
            masks.make_identity(nc, ident[64:128, :])
            ones_col = cons.tile([128, 1], bf16)
            nc.vector.memset(ones_col[:], 1.0)
            onesp8 = cons.tile([128, 2, 128], fp8)  # DR colsum lhsT
            nc.vector.memset(onesp8[:], 1.0)
            inv16_col = cons.tile([1, 128], bf16)   # bcast lhsT: 1/SW
            nc.vector.memset(inv16_col[:], 1.0 / SW)
            s32_row = cons.tile([1, 64], bf16)      # softmax bcast lhsT: SCTX
            nc.vector.memset(s32_row[:], SCTX)
            sel = cons.tile([16, 2], fp8)           # stats combiner
            nc.sync.dma_start(sel[:], seli[:])

            ncsq_f = cons.tile([1, NQKV], f32)
            nc.sync.dma_start(ncsq_f[:], ncsq[:])
            ncsq_row = cons.tile([1, NQKV], bf16)
            nc.vector.tensor_copy(ncsq_row[:], ncsq_f[:])
            ncs1_f = cons.tile([1, 512], f32)
            nc.sync.dma_start(ncs1_f[:], ncs1[:])
            ncs1_row = cons.tile([1, 512], bf16)
            nc.vector.tensor_copy(ncs1_row[:], ncs1_f[:])

            # ---------------- persistent SBUF ----------------
            wqkv_sb = wp.tile([128, DC, NQKV], fp8, name="wqkv_sb")
            nc.sync.dma_start(wqkv_sb[:], wqkv[:])
            ow_sb = wp.tile([128, D], bf16, name="ow_sb")
            nc.sync.dma_start(ow_sb[:], ow[:])

            qkvT = [qkvp.tile([128, T], bf16, name=f"qkvT{n}") for n in range(3)]
            ctxT = qkvp.tile([128, T], bf16, name="ctxT")

            # LN row buffers (3D views: [1, n, 16] <=> n*16 tokens)
            rstd1_all = wp.tile([1, 256, 16], bf16, name="rstd1_all")
            m1_all = wp.tile([1, T], bf16, name="m1_all")
            rstd2_all = [wp.tile([1, 128, 16], bf16, name=f"rstd2_all{b}")
                         for b in range(B)]
            m2_all = [wp.tile([1, S], bf16, name=f"m2_all{b}") for b in range(B)]

            # ---------------- DRAM scratch ----------------
            ar_in = [dram.tile([D, S], fp8, name=f"ar_in{b}") for b in range(B)]
            ag_in = [dram.tile([130, S], fp8, name=f"ag_in{b}") for b in range(B)]
            ag_out = [dram.tile([NC, 130, S], fp8, name=f"ag_out{b}",
                                addr_space="Shared") for b in range(B)]
            rs2_in = [dram.tile([D, S // 2], fp8, name=f"rs2_in{b}{h}")
                      for b in range(B) for h in range(2)]
            rs2_out = [dram.tile([128, S // 2], fp8, name=f"rs2_out{b}{h}")
                       for b in range(B) for h in range(2)]

            drain_rr = [0]

            def drain(dst, src):
                """PSUM -> SBUF drain, alternating vector/scalar engines."""
                drain_rr[0] ^= 1
                if drain_rr[0]:
                    nc.vector.tensor_copy(dst, src)
                else:
                    nc.scalar.activation(dst, src, AF.Copy)

            # =========== P1: LN1 stats (batched, Sqrt-only on ACT) =========
            def p1_stats(blk):
                """stats chains for one 512-token block -> variance row."""
                tsl = slice(512 * blk, 512 * (blk + 1))
                xb = xbp.tile([128, DC, 512], fp8, tag="xb", name="xb")
                nc.sync.dma_start(xb[:], xp[:, :, tsl])
                sq = [sqp.tile([128, 2, 512], fp8, tag="sq", name="sq")
                      for _ in range(4)]
                for p in range(4):
                    nc.gpsimd.tensor_tensor(sq[p][:], xb[:, 2 * p:2 * p + 2, :],
                                            xb[:, 2 * p:2 * p + 2, :],
                                            op=ALU.mult)
                srow = psA.tile([128, 512], f32, tag="acc", name="srow")
                qrow = psA.tile([128, 512], f32, tag="acc", name="qrow")
                for p in range(4):
                    nc.tensor.matmul(srow[:], onesp8[:],
                                     xb[:, 2 * p:2 * p + 2, :],
                                     start=(p == 0), stop=(p == 3), perf_mode=DR)
                for p in range(4):
                    nc.tensor.matmul(qrow[:], onesp8[:], sq[p][:],
                                     start=(p == 0), stop=(p == 3), perf_mode=DR)
                m1 = m1_all[0:1, tsl]
                nc.vector.tensor_copy(m1, srow[0:1, :])
                msq = rowp.tile([1, 512], f32, tag="row", name="msq")
                nc.vector.scalar_tensor_tensor(msq[:], m1, 1.0 / (D * D), m1,
                                               op0=ALU.mult, op1=ALU.mult)
                vv = rowp.tile([1, 512], f32, tag="row", name="vv")
                nc.vector.scalar_tensor_tensor(vv[:], qrow[0:1, :], 1.0 / D,
                                               msq[:], op0=ALU.mult,
                                               op1=ALU.subtract)
                return vv

            def rstd_dance(vv, dst3d):
                """vv [1,512] variance row -> 1/sqrt in [1,32,16] bf16 slice.

                Reshape to [128,4] via DMA so DVE's iterative reciprocal runs
                128 lanes wide, then Sqrt(1/v) on Scalar (kept adjacent with
                the other dances so the sqrt table loads once)."""
                dcol = rowp.tile([128, 4], f32, tag="dcol4", name="dcolv")
                nc.sync.dma_start(dcol[:], vv[:])
                rcp = rowp.tile([128, 4], f32, tag="rcp4", name="rcpv")
                nc.vector.reciprocal(rcp[:], dcol[:])
                rst = rowp.tile([128, 4], bf16, tag="rst4", name="rstv")
                nc.scalar.activation(rst[:], rcp[:], AF.Sqrt)
                nc.sync.dma_start(dst3d, rst[:])

            def p1_qkv(blk):
                tsl = slice(512 * blk, 512 * (blk + 1))
                xb = xbp.tile([128, DC, 512], fp8, tag="xb", name="xbq")
                nc.sync.dma_start(xb[:], xp[:, :, tsl])
                rps = psB.tile([128, 512], f32, tag="bc", name="rps")
                nc.tensor.matmul(rps[:], inv16_col[:],
                                 rstd1_all[0:1, 32 * blk:32 * (blk + 1), :],
                                 start=True, stop=True)
                rbc = gip.tile([128, 512], f32, tag="bcs", name="rbc")
                nc.scalar.activation(rbc[:], rps[:], AF.Copy)
                for n in range(3):
                    qps = psA.tile([128, 512], f32, tag="acc", name="qps")
                    for p in range(4):
                        nc.tensor.matmul(
                            qps[:], wqkv_sb[:, 2 * p:2 * p + 2,
                                            128 * n:128 * (n + 1)],
                            xb[:, 2 * p:2 * p + 2, :],
                            start=(p == 0), stop=False, perf_mode=DR)
                    nc.tensor.matmul(qps[:],
                                     ncsq_row[0:1, 128 * n:128 * (n + 1)],
                                     m1_all[0:1, tsl], start=False, stop=True)
                    nc.vector.tensor_tensor(qkvT[n][:, tsl], qps[:], rbc[:],
                                            op=ALU.mult)

            # =========== P2 attention pieces ===========
            def attn_vaug(b, h):
                hb = 64 * h
                bsl0 = S * b
                vaug = []
                for kp in range(8):           # kc pairs
                    va = vaugp.tile([128, 2, 128], fp8, tag="vaug", name="va")
                    nc.vector.memset(va[:, :, 64:65], 1.0)
                    nc.vector.memset(va[:, :, 65:128], 0.0)
                    for j in range(2):
                        kc = 2 * kp + j
                        tp = psT.tile([128, 64], bf16, tag="tp", name="tp")
                        nc.tensor.transpose(
                            tp[:], qkvT[2][hb:hb + 64,
                                           bsl0 + 128 * kc:bsl0 + 128 * (kc + 1)],
                            ident[hb:hb + 64, :])
                        nc.vector.tensor_copy(va[:, j, 0:64], tp[:])
                    vaug.append(va)
                return vaug

            def attn_qc(b, h, qc, vaug):
                hb = 64 * h
                bsl0 = S * b
                qsl = qkvT[0][hb:hb + 64, bsl0 + 512 * qc:bsl0 + 512 * (qc + 1)]
                exps = []
                for r in range(4):            # rounds of 4 kc
                    sc = psS.tile([128, 4, 512], f32, tag="sc", name="sc")
                    for j in range(4):
                        kc = 4 * r + j
                        nc.tensor.matmul(
                            sc[:, j, :],
                            qkvT[1][hb:hb + 64,
                                    bsl0 + 128 * kc:bsl0 + 128 * (kc + 1)],
                            qsl, start=True, stop=True)
                    e = expp.tile([128, 4, 512], fp8, tag="exp", name="e")
                    nc.scalar.activation(e[:], sc[:], AF.Exp)
                    exps.append(e)
                cps = psA.tile([128, 512], f32, tag="acc", name="cps")
                for r in range(4):
                    for j in range(2):
                        nc.tensor.matmul(cps[:], vaug[2 * r + j][:],
                                         exps[r][:, 2 * j:2 * j + 2, :],
                                         start=(r == 0 and j == 0),
                                         stop=(r == 3 and j == 1), perf_mode=DR)
                # denominator reciprocal via DVE (no ACT table involved)
                drow = rowp.tile([1, 128, 4], f32, tag="drow", name="drow")
                nc.vector.tensor_copy(drow[0:1, :, :], cps[64:65, :])
                dcol = rowp.tile([128, 4], f32, tag="dcol4", name="dcol4")
                nc.sync.dma_start(dcol[:], drow[0:1, :, :])
                dcr = rowp.tile([128, 4], bf16, tag="dcr4", name="dcr4")
                with nc.allow_low_precision(reason="bf16 softmax denom ok"):
                    nc.vector.reciprocal(dcr[:], dcol[:])
                rrow = rowp.tile([1, 128, 4], bf16, tag="rrow", name="rrow")
                nc.sync.dma_start(rrow[0:1, :, :], dcr[:])
                rb = psB.tile([128, 512], f32, tag="bc", name="rb")
                nc.tensor.matmul(rb[0:64, :], s32_row[:], rrow[0:1, :, :],
                                 start=True, stop=True)
                rbs = gip.tile([128, 512], f32, tag="bcs", name="rbs")
                nc.scalar.activation(rbs[0:64, :], rb[0:64, :], AF.Copy)
                cn = drp.tile([64, 512], bf16, tag="cn", name="cn")
                nc.vector.tensor_tensor(cn[:], cps[0:64, :], rbs[0:64, :],
                                        op=ALU.mult)
                nc.sync.dma_start(
                    ctxT[hb:hb + 64, bsl0 + 512 * qc:bsl0 + 512 * (qc + 1)],
                    cn[:])

            # =========== P3: attn-out partials ===========
            def p3(b):
                bsl0 = S * b
                for tcc in range(4):
                    for oc in range(DC):
                        ops3 = psA.tile([128, 512], f32, tag="acc", name="ops3")
                        nc.tensor.matmul(
                            ops3[:], ow_sb[:, 128 * oc:128 * (oc + 1)],
                            ctxT[:, bsl0 + 512 * tcc:bsl0 + 512 * (tcc + 1)],
                            start=True, stop=True)
                        po = drp.tile([128, 512], fp8, tag="po", name="po")
                        drain(po[:], ops3[:])
                        nc.sync.dma_start(
                            ar_in[b][128 * oc:128 * (oc + 1),
                                     512 * tcc:512 * (tcc + 1)], po[:])

            # =========== stripe stats -> AG payload ===========
            def stripe_stats(b):
                for tcc in range(4):
                    tsl = slice(512 * tcc, 512 * (tcc + 1))
                    gsl = slice(S * b + 512 * tcc, S * b + 512 * (tcc + 1))
                    rs_t = wfp.tile([128, 512], fp8, tag="rs8", name="rs_t")
                    nc.sync.dma_start(rs_t[:], ag_in[b][0:128, tsl])
                    xot = wfp.tile([128, 512], f32, tag="xot", name="xot")
                    nc.sync.dma_start(xot[:], x_own[:, gsl])
                    rsc = wfp.tile([128, 512], f32, tag="wf", name="rsc")
                    nc.gpsimd.tensor_scalar(rsc[:], rs_t[:], 1.0 / SAR, None,
                                            op0=ALU.mult)
                    rof = wfp.tile([128, 512], f32, tag="wf", name="rof")
                    nc.gpsimd.tensor_tensor(rof[:], rsc[:], xot[:], op=ALU.add)
                    rof_bf = wfp.tile([128, 512], bf16, tag="wfb", name="rof_bf")
                    nc.gpsimd.tensor_copy(rof_bf[:], rof[:])
                    sqf = wfp.tile([128, 512], bf16, tag="wfb", name="sqf")
                    nc.gpsimd.tensor_tensor(sqf[:], rof[:], rof[:], op=ALU.mult)
                    srow = psA.tile([128, 512], f32, tag="acc", name="ssrow")
                    nc.tensor.matmul(srow[0:1, :], ones_col[:], rof_bf[:],
                                     start=True, stop=True)
                    qrow = psA.tile([128, 512], f32, tag="acc", name="sqrow")
                    nc.tensor.matmul(qrow[0:1, :], ones_col[:], sqf[:],
                                     start=True, stop=True)
                    st8a = rowp.tile([1, 512], fp8, tag="st8", name="st8a")
                    nc.vector.tensor_scalar(st8a[:], srow[0:1, :],
                                            1.0 / SAR, None, op0=ALU.mult)
                    st8b = rowp.tile([1, 512], fp8, tag="st8", name="st8b")
                    nc.vector.tensor_scalar(st8b[:], qrow[0:1, :],
                                            1.0 / SAR, None, op0=ALU.mult)
                    nc.sync.dma_start(ag_in[b][128:129, tsl], st8a[:])
                    nc.sync.dma_start(ag_in[b][129:130, tsl], st8b[:])

            # =========== P4 stats (batched per b, after AG) ===========
            def p4_stats(b):
                st = sqp.tile([16, S], fp8, tag="st", name="st", bufs=1)
                nc.sync.dma_start(st[:], ag_out[b][0:NC, 128:130, :])
                for tcc in range(4):
                    tsl = slice(512 * tcc, 512 * (tcc + 1))
                    tot = psA.tile([128, 512], f32, tag="acc", name="tot")
                    nc.tensor.matmul(tot[0:1, :], sel[:, 0:1], st[:, tsl],
                                     start=True, stop=True)
                    totq = psA.tile([128, 512], f32, tag="acc", name="totq")
                    nc.tensor.matmul(totq[0:1, :], sel[:, 1:2], st[:, tsl],
                                     start=True, stop=True)
                    m2 = m2_all[b][0:1, tsl]
                    nc.vector.tensor_copy(m2, tot[0:1, :])
                    msq = rowp.tile([1, 512], f32, tag="row", name="msq2")
                    nc.vector.scalar_tensor_tensor(
                        msq[:], m2, 1.0 / (SW * SW), m2,
                        op0=ALU.mult, op1=ALU.mult)
                    vv = rowp.tile([1, 512], f32, tag="row", name="vv2")
                    nc.vector.scalar_tensor_tensor(
                        vv[:], totq[0:1, :], 1.0 / SW, msq[:],
                        op0=ALU.mult, op1=ALU.subtract)
                    rstd_dance(vv, rstd2_all[b][0:1, 32 * tcc:32 * (tcc + 1), :])

            # =========== P4: MLP block (one 512-token chunk) ===========
            def p4(b, tcc):
                tsl = slice(512 * tcc, 512 * (tcc + 1))
                gsl = slice(S * b + 512 * tcc, S * b + 512 * (tcc + 1))
                ag_t, rs_t = [], []
                for p in range(4):
                    ag = agp.tile([128, 2, 512], fp8, tag="ag", name="ag")
                    for j in range(2):
                        d = 2 * p + j
                        nc.sync.dma_start(ag[:, j, :],
                                          ag_out[b][d:d + 1, 0:128, tsl])
                    xpr = rsdp.tile([128, 2, 512], fp8, tag="xpr", name="xpr")
                    nc.sync.dma_start(xpr[:], xp[:, 2 * p:2 * p + 2, gsl])
                    rsd = rsdp.tile([128, 2, 512], fp8, tag="rsd", name="rsd")
                    nc.vector.scalar_tensor_tensor(
                        rsd[:], ag[:], 1.0 / SAR, xpr[:],
                        op0=ALU.mult, op1=ALU.add)
                    ag_t.append(ag)
                    rs_t.append(rsd)

                rps = psB.tile([128, 512], f32, tag="bc", name="rps2")
                nc.tensor.matmul(rps[:], inv16_col[:],
                                 rstd2_all[b][0:1, 32 * tcc:32 * (tcc + 1), :],
                                 start=True, stop=True)
                rbc = gip.tile([128, 512], f32, tag="bcs", name="rbc2")
                nc.scalar.activation(rbc[:], rps[:], AF.Copy)

                it_t = [itp.tile([128, 2, 512], fp8, tag="it", name="it")
                        for _ in range(2)]
                for ic in range(4):
                    h2ps = psA.tile([128, 512], f32, tag="acc", name="h2ps")
                    for p in range(4):
                        nc.tensor.matmul(
                            h2ps[:], w2_sb[:, 2 * p:2 * p + 2,
                                           128 * ic:128 * (ic + 1)],
                            ag_t[p][:], start=(p == 0), stop=(p == 3),
                            perf_mode=DR)
                    h1ps = psA.tile([128, 512], f32, tag="acc", name="h1ps")
                    for p in range(4):
                        nc.tensor.matmul(
                            h1ps[:], w1_sb[:, 2 * p:2 * p + 2,
                                           128 * ic:128 * (ic + 1)],
                            rs_t[p][:], start=(p == 0), stop=False,
                            perf_mode=DR)
                    nc.tensor.matmul(h1ps[:],
                                     ncs1_row[0:1, 128 * ic:128 * (ic + 1)],
                                     m2_all[b][0:1, tsl], start=False, stop=True)
                    gi = gip.tile([128, 512], f32, tag="gi", name="gi")
                    nc.vector.tensor_tensor(gi[:], h1ps[:], rbc[:], op=ALU.mult)
                    h1 = h1p.tile([128, 512], bf16, tag="h1", name="h1")
                    nc.scalar.activation(h1[:], gi[:],
                                         AF.Sigmoid if SIM_GELU else AF.Gelu)
                    nc.vector.scalar_tensor_tensor(
                        it_t[ic // 2][:, ic % 2, :], h2ps[:], 1.0 / SW, h1[:],
                        op0=ALU.mult, op1=ALU.mult)
                for oc in range(DC):
                    ops3 = psA.tile([128, 512], f32, tag="acc", name="ops4")
                    for icp in range(2):
                        nc.tensor.matmul(
                            ops3[:], outw_sb[:, 2 * icp:2 * icp + 2,
                                             128 * oc:128 * (oc + 1)],
                            it_t[icp][:], start=(icp == 0), stop=(icp == 1),
                            perf_mode=DR)
                    po2 = drp.tile([128, 512], fp8, tag="po", name="po2")
                    drain(po2[:], ops3[:])
                    nc.sync.dma_start(
                        rs2_in[2 * b + tcc // 2][128 * oc:128 * (oc + 1),
                                                 512 * (tcc % 2):
                                                 512 * (tcc % 2 + 1)], po2[:])

            # =========== P6: final stripe chunks ===========
            def p6(b, tcc):
                tsl = slice(512 * tcc, 512 * (tcc + 1))
                gsl = slice(S * b + 512 * tcc, S * b + 512 * (tcc + 1))
                rs_t = wfp.tile([128, 512], fp8, tag="rs8", name="rs_t6")
                nc.sync.dma_start(rs_t[:], ag_in[b][0:128, tsl])
                xot = wfp.tile([128, 512], f32, tag="xot", name="xot6")
                nc.sync.dma_start(xot[:], x_own[:, gsl])
                rsc = wfp.tile([128, 512], f32, tag="wf", name="rsc6")
                nc.gpsimd.tensor_scalar(rsc[:], rs_t[:], 1.0 / SAR, None,
                                        op0=ALU.mult)
                t1 = wfp.tile([128, 512], f32, tag="wf", name="t1")
                nc.gpsimd.tensor_tensor(t1[:], rsc[:], xot[:], op=ALU.add)
                r2 = wfp.tile([128, 512], fp8, tag="rs8", name="r2")
                nc.sync.dma_start(
                    r2[:], rs2_out[2 * b + tcc // 2][:, 512 * (tcc % 2):
                                                     512 * (tcc % 2 + 1)])
                r2c = wfp.tile([128, 512], f32, tag="wf", name="r2c")
                nc.gpsimd.tensor_scalar(r2c[:], r2[:], 1.0 / SO2, None,
                                        op0=ALU.mult)
                ot = wfp.tile([128, 512], f32, tag="wf", name="ot")
                nc.gpsimd.tensor_tensor(ot[:], r2c[:], t1[:], op=ALU.add)
                nc.sync.dma_start(outT[:, gsl], ot[:])

            # ================= EMISSION =================
            # LN1 stats; p1_stats touches no ACT, so the per-block Sqrt
            # dances stay on one activation-table set
            for blk in range(8):
                vv = p1_stats(blk)
                rstd_dance(vv, rstd1_all[0:1, 32 * blk:32 * (blk + 1), :])
            # QKV for batch-0 tokens
            for blk in range(4):
                p1_qkv(blk)

            # MLP weights now (DMA overlaps attention)
            w1_sb = wp.tile([128, DC, 512], fp8, name="w1_sb")
            nc.sync.dma_start(w1_sb[:], w1[:])
            w2_sb = wp.tile([128, DC, 512], fp8, name="w2_sb")
            nc.sync.dma_start(w2_sb[:], w2[:])
            outw_sb = wp.tile([128, 4, D], fp8, name="outw_sb")
            nc.sync.dma_start(outw_sb[:], outw[:])

            # attention(b0) interleaved with QKV blocks 4..7 (batch-1 tokens)
            rem = [4, 5, 6, 7]
            for h in range(2):
                vaug = attn_vaug(0, h)
                for qc in range(4):
                    if rem:
                        p1_qkv(rem.pop(0))
                    attn_qc(0, h, qc, vaug)

            p3(0)
            nc.gpsimd.collective_compute(
                "ReduceScatter", ALU.add, ins=[ar_in[0].opt()],
                outs=[ag_in[0][0:128, :].opt()], replica_groups=RG)

            # attention(b1) fully covers RS(b0) latency
            for h in range(2):
                vaug = attn_vaug(1, h)
                for qc in range(4):
                    attn_qc(1, h, qc, vaug)

            stripe_stats(0)
            nc.gpsimd.collective_compute(
                "AllGather", ALU.bypass, ins=[ag_in[0].opt()],
                outs=[ag_out[0].opt()], replica_groups=RG)

            p3(1)
            nc.gpsimd.collective_compute(
                "ReduceScatter", ALU.add, ins=[ar_in[1].opt()],
                outs=[ag_in[1][0:128, :].opt()], replica_groups=RG)

            p4_stats(0)
            p4(0, 0)
            p4(0, 1)
            stripe_stats(1)
            nc.gpsimd.collective_compute(
                "AllGather", ALU.bypass, ins=[ag_in[1].opt()],
                outs=[ag_out[1].opt()], replica_groups=RG)
            nc.gpsimd.collective_compute(
                "ReduceScatter", ALU.add, ins=[rs2_in[0].opt()],
                outs=[rs2_out[0].opt()], replica_groups=RG)
            p4(0, 2)
            p4(0, 3)
            p4_stats(1)
            nc.gpsimd.collective_compute(
                "ReduceScatter", ALU.add, ins=[rs2_in[1].opt()],
                outs=[rs2_out[1].opt()], replica_groups=RG)
            p4(1, 0)
            p4(1, 1)
            nc.gpsimd.collective_compute(
                "ReduceScatter", ALU.add, ins=[rs2_in[2].opt()],
                outs=[rs2_out[2].opt()], replica_groups=RG)
            p6(0, 0)
            p6(0, 1)
            p4(1, 2)
            p4(1, 3)
            nc.gpsimd.collective_compute(
                "ReduceScatter", ALU.add, ins=[rs2_in[3].opt()],
                outs=[rs2_out[3].opt()], replica_groups=RG)
            p6(0, 2)
            p6(0, 3)
            for tcc in range(4):
                p6(1, tcc)

    nc.compile()
    return nc


_NC_CACHE = {}


def make_in_maps(**inputs):
    x = np.asarray(inputs["x"], np.float32)
    norm_w = np.asarray(inputs["norm_w"], np.float32)
    norm_b = np.asarray(inputs["norm_b"], np.float32)
    qkvw = np.asarray(inputs["attn_qkvw"], np.float32)
    qkvb = np.asarray(inputs["attn_qkvb"], np.float32)
    attn_ow = np.asarray(inputs["attn_ow"], np.float32)
    attn_ob = np.asarray(inputs["attn_ob"], np.float32)
    attn_nw = np.asarray(inputs["attn_nw"], np.float32)
    attn_nb = np.asarray(inputs["attn_nb"], np.float32)
    inter_w = np.asarray(inputs["inter_w"], np.float32)
    inter_b = np.asarray(inputs["inter_b"], np.float32)
    inter_w1 = np.asarray(inputs["inter_w1"], np.float32)
    output_w = np.asarray(inputs["output_w"], np.float32)
    output_b = np.asarray(inputs["output_b"], np.float32)

    X = x.reshape(T, D)
    XT = np.ascontiguousarray(X.T)              # [D, T]

    # LN folds
    wqkv_f = norm_w[:, None] * qkvw
    bqkv_f = qkvb + norm_b @ qkvw
    wqkv_f = wqkv_f.copy()
    wqkv_f[:, :D] /= np.sqrt(HD)
    w1_f = attn_nw[:, None] * inter_w
    b1_f = inter_b + attn_nb @ inter_w

    assert not np.any(bqkv_f) and not np.any(attn_ob) and not np.any(b1_f) \
        and not np.any(output_b), "nonzero biases not wired in this build"

    xp_all = _f8(_planes(XT))                   # x planes [128, 8, T]

    in_maps = []
    for c in range(NC):
        hsl = slice(128 * c, 128 * (c + 1))
        isl = slice(512 * c, 512 * (c + 1))
        wq_c = np.concatenate(
            [wqkv_f[:, hsl], wqkv_f[:, D:][:, hsl], wqkv_f[:, 2 * D:][:, hsl]],
            axis=1)                             # [D, 384]
        wq8 = _f8(_planes(wq_c * SW))
        w1_c = w1_f[:, isl]
        w18 = _f8(_planes(w1_c * SW))
        w28 = _f8(_planes(inter_w1[:, isl] * (SCTX * SW / SAR)))
        ou8 = _f8(output_w[isl, :].reshape(4, 128, D).transpose(1, 0, 2)
                  * (SO2 / SCTX))
        wq_deq = wq8.astype(np.float32).transpose(1, 0, 2).reshape(D, NQKV)
        w1_deq = w18.astype(np.float32).transpose(1, 0, 2).reshape(D, 512)
        in_maps.append({
            "xp": xp_all,
            "x_own": np.ascontiguousarray(XT[hsl, :]),
            "wqkv": wq8,
            "ncsq": np.ascontiguousarray(-wq_deq.sum(0, keepdims=True) / D),
            "ow": _bf(attn_ow[hsl, :] * (SAR / SCTX)),
            "w1": w18,
            "ncs1": np.ascontiguousarray(
                -w1_deq.sum(0, keepdims=True) / SW),
            "w2": w28,
            "outw": np.ascontiguousarray(ou8),
            "seli": _f8(np.tile(np.eye(2, dtype=np.float32), (8, 1))),
        })
    return in_maps


def kernel(**inputs):
    if "nc" not in _NC_CACHE:
        _NC_CACHE["nc"] = _build()
    nc = _NC_CACHE["nc"]
    in_maps = make_in_maps(**inputs)
    global _LAST_IN_MAPS
    _LAST_IN_MAPS = in_maps
    res = run_bass_kernel_spmd(nc, in_maps, list(range(NC)))
    outT = np.concatenate([res.results[c]["outT"] for c in range(NC)], axis=0)
    return np.ascontiguousarray(outT.T).reshape(B, S, D).astype(np.float32)


if __name__ == "__main__":
    pass


# revision 35
# speedup vs baseline: 1.2020x; 1.2020x over previous
"""Tensor-parallel DeepSpeed encoder-decoder block on 8 TRN2 NeuronCores.

Sharding (mp_group scheme): attn_qkvw / inter_w / inter_w1 column-sharded
(2 heads / 512 intermediate cols per core), attn_ow / output_w row-sharded.
Post-attn all-reduce = ReduceScatter + AllGather (fp8 payloads, x64 scale
folded host-side); post-output_w all-reduce = ReduceScatter only (fp8, x256
scale), each core finishing its own 128-row feature stripe.

Device compute: fp8(e4m3) DoubleRow matmuls (K=256 per instruction) for the
QKV / MLP / probs-V GEMMs; bf16 for scores and attn-out. LayerNorms fold
gamma/beta into weights host-side; mean correction is a rank-1 matmul; the
1/std factor is Sqrt on Scalar + reciprocal on DVE (rows reshaped to
[128,N] via SBUF->SBUF DMA so the iterative divide runs 128 lanes wide),
broadcast via a K=1 matmul and applied at PSUM drain.  All Sqrt calls are
batched so the Scalar engine's activation-table set switches only a handful
of times across the whole kernel (Exp for attention, Sqrt for the two LN
stats batches, Gelu for the MLP).  LN2 stats ride inside the AllGather as 2
extra rows per rank (per-stripe sum / sum-of-squares of the residual), so
no extra collective and no second pass over the activations.  Softmax runs
in transposed layout with the denominator produced by a ones-column
augmentation of V; its reciprocal also takes the DVE [128,N] path.  GpSimd
handles collective triggers plus pre-collective elementwise work only, so
compute queues never block on collective waits.
"""
from contextlib import ExitStack

import numpy as np
import ml_dtypes

import concourse.bacc as bacc
import concourse.mybir as mybir
import concourse.tile as tile
from concourse import masks
from concourse.bass_utils import run_bass_kernel_spmd

f32 = mybir.dt.float32
bf16 = mybir.dt.bfloat16
fp8 = mybir.dt.float8e4
AF = mybir.ActivationFunctionType
ALU = mybir.AluOpType
DR = mybir.MatmulPerfMode.DoubleRow

NC = 8
B, S, D, I = 2, 2048, 1024, 4096
H, HD = 16, 64
T = B * S
DC = D // 128          # 8 feature chunks
NQKV = 384             # qkv cols per core
EPS = 1e-12

SIM_GELU = False       # sim doesn't implement Gelu; swap for Sigmoid there

SW = 16.0              # fp8 weight scale (wqkv, w1, w2, outw)
SCTX = 32.0            # ctxT scale
SAR = 64.0             # attn partial / RS / AG scale
SO2 = 256.0            # mlp partial / RS2 scale

_BF = ml_dtypes.bfloat16
_F8 = ml_dtypes.float8_e4m3fn


def _bf(a):
    return np.ascontiguousarray(np.asarray(a, np.float32).astype(_BF))


def _f8(a):
    return np.ascontiguousarray(
        np.clip(np.asarray(a, np.float32), -240.0, 240.0).astype(_F8))


def _planes(w):  # [D, F] -> [128, D//128, F]
    w = np.asarray(w, np.float32)
    return w.reshape(DC, 128, w.shape[1]).transpose(1, 0, 2)


def _build():
    nc = bacc.Bacc("TRN2", target_bir_lowering=False, debug=False,
                   num_devices=NC)

    inp = {}

    def din(name, shape, dt):
        inp[name] = nc.dram_tensor(name, shape, dt, kind="ExternalInput")
        return inp[name]

    xp = din("xp", [128, DC, T], fp8)          # x feature-planes
    x_own = din("x_own", [128, T], f32)        # core's 128-feat stripe of x
    wqkv = din("wqkv", [128, DC, NQKV], fp8)   # 16x folded-LN1 qkv weights
    ncsq = din("ncsq", [1, NQKV], f32)         # -colsum(wqkv_dev)/D
    ow = din("ow", [128, D], bf16)             # 2x attn_ow rows
    w1 = din("w1", [128, DC, 512], fp8)        # 16x folded-LN2 inter_w
    ncs1 = din("ncs1", [1, 512], f32)          # -colsum(w1_dev)/SW
    w2 = din("w2", [128, DC, 512], fp8)        # inter_w1 / 4
    outw = din("outw", [128, 4, D], fp8)       # 8x output_w
    seli = din("seli", [16, 2], fp8)           # LN2 stats combiner

    outT = nc.dram_tensor("outT", [128, T], f32, kind="ExternalOutput")

    RG = [list(range(NC))]

    with tile.TileContext(nc) as tc:
        with ExitStack() as ctx:
            ep = ctx.enter_context
            cons = ep(tc.tile_pool(name="cons", bufs=1))
            wp = ep(tc.tile_pool(name="wp", bufs=1))
            qkvp = ep(tc.tile_pool(name="qkvp", bufs=1))
            rowp = ep(tc.tile_pool(name="rowp", bufs=3))
            sqp = ep(tc.tile_pool(name="sqp", bufs=3))
            xbp = ep(tc.tile_pool(name="xbp", bufs=3))
            vaugp = ep(tc.tile_pool(name="vaugp", bufs=9))
            expp = ep(tc.tile_pool(name="expp", bufs=4))
            drp = ep(tc.tile_pool(name="drp", bufs=4))
            agp = ep(tc.tile_pool(name="agp", bufs=5))
            rsdp = ep(tc.tile_pool(name="rsdp", bufs=5))
            gip = ep(tc.tile_pool(name="gip", bufs=2))
            h1p = ep(tc.tile_pool(name="h1p", bufs=2))
            itp = ep(tc.tile_pool(name="itp", bufs=4))
            wfp = ep(tc.tile_pool(name="wfp", bufs=5))
            # PSUM banks: psA 2 + psB 1 + psS 4 + psT 1 = 8
            psA = ep(tc.tile_pool(name="psA", bufs=2, space="PSUM"))
            psB = ep(tc.tile_pool(name="psB", bufs=1, space="PSUM"))
            psS = ep(tc.tile_pool(name="psS", bufs=1, space="PSUM"))
            psT = ep(tc.tile_pool(name="psT", bufs=1, space="PSUM"))
            dram = ep(tc.tile_pool(name="dram", bufs=1, space="DRAM"))

            # ---------------- constants ----------------
            ident = cons.tile([128, 64], bf16)
            masks.make_identity(nc, ident[0:64, :])
            masks.make_identity(nc, ident[64:128, :])
            ones_col = cons.tile([128, 1], bf16)
            nc.vector.memset(ones_col[:], 1.0)
            onesp8 = cons.tile([128, 2, 128], fp8)  # DR colsum lhsT
            nc.vector.memset(onesp8[:], 1.0)
            inv16_col = cons.tile([1, 128], bf16)   # bcast lhsT: 1/SW
            nc.vector.memset(inv16_col[:], 1.0 / SW)
            s32_row = cons.tile([1, 64], bf16)      # softmax bcast lhsT: SCTX
            nc.vector.memset(s32_row[:], SCTX)
            sel = cons.tile([16, 2], fp8)           # stats combiner
            nc.sync.dma_start(sel[:], seli[:])

            ncsq_f = cons.tile([1, NQKV], f32)
            nc.sync.dma_start(ncsq_f[:], ncsq[:])
            ncsq_row = cons.tile([1, NQKV], bf16)
            nc.vector.tensor_copy(ncsq_row[:], ncsq_f[:])
            ncs1_f = cons.tile([1, 512], f32)
            nc.sync.dma_start(ncs1_f[:], ncs1[:])
            ncs1_row = cons.tile([1, 512], bf16)
            nc.vector.tensor_copy(ncs1_row[:], ncs1_f[:])

            # ---------------- persistent SBUF ----------------
            wqkv_sb = wp.tile([128, DC, NQKV], fp8, name="wqkv_sb")
            nc.sync.dma_start(wqkv_sb[:], wqkv[:])
            ow_sb = wp.tile([128, D], bf16, name="ow_sb")
            nc.sync.dma_start(ow_sb[:], ow[:])

            qkvT = [qkvp.tile([128, T], bf16, name=f"qkvT{n}") for n in range(3)]
            ctxT = qkvp.tile([128, T], bf16, name="ctxT")

            # LN row buffers (3D views: [1, n, 16] <=> n*16 tokens)
            rstd1_all = wp.tile([1, 256, 16], bf16, name="rstd1_all")
            m1_all = wp.tile([1, T], bf16, name="m1_all")
            rstd2_all = [wp.tile([1, 128, 16], bf16, name=f"rstd2_all{b}")
                         for b in range(B)]
            m2_all = [wp.tile([1, S], bf16, name=f"m2_all{b}") for b in range(B)]

            # ---------------- DRAM scratch ----------------
            ar_in = [dram.tile([D, S], fp8, name=f"ar_in{b}") for b in range(B)]
            ag_in = [dram.tile([130, S], fp8, name=f"ag_in{b}") for b in range(B)]
            ag_out = [dram.tile([NC, 130, S], fp8, name=f"ag_out{b}",
                                addr_space="Shared") for b in range(B)]
            rs2_in = [dram.tile([D, S // 2], fp8, name=f"rs2_in{b}{h}")
                      for b in range(B) for h in range(2)]
            rs2_out = [dram.tile([128, S // 2], fp8, name=f"rs2_out{b}{h}")
                       for b in range(B) for h in range(2)]

            drain_rr = [0]

            def drain(dst, src):
                """PSUM -> SBUF drain, alternating vector/scalar engines."""
                drain_rr[0] ^= 1
                if drain_rr[0]:
                    nc.vector.tensor_copy(dst, src)
                else:
                    nc.scalar.activation(dst, src, AF.Copy)

            # =========== P1: LN1 stats (batched, Sqrt-only on ACT) =========
            def p1_stats(blk):
                """stats chains for one 512-token block -> variance row."""
                tsl = slice(512 * blk, 512 * (blk + 1))
                xb = xbp.tile([128, DC, 512], fp8, tag="xb", name="xb")
                nc.sync.dma_start(xb[:], xp[:, :, tsl])
                sq = [sqp.tile([128, 2, 512], fp8, tag="sq", name="sq")
                      for _ in range(4)]
                for p in range(4):
                    nc.vector.tensor_tensor(sq[p][:], xb[:, 2 * p:2 * p + 2, :],
                                            xb[:, 2 * p:2 * p + 2, :],
                                            op=ALU.mult)
                srow = psA.tile([128, 512], f32, tag="acc", name="srow")
                qrow = psA.tile([128, 512], f32, tag="acc", name="qrow")
                for p in range(4):
                    nc.tensor.matmul(srow[:], onesp8[:],
                                     xb[:, 2 * p:2 * p + 2, :],
                                     start=(p == 0), stop=(p == 3), perf_mode=DR)
                for p in range(4):
                    nc.tensor.matmul(qrow[:], onesp8[:], sq[p][:],
                                     start=(p == 0), stop=(p == 3), perf_mode=DR)
                m1 = m1_all[0:1, tsl]
                nc.vector.tensor_copy(m1, srow[0:1, :])
                msq = rowp.tile([1, 512], f32, tag="row", name="msq")
                nc.vector.scalar_tensor_tensor(msq[:], m1, 1.0 / (D * D), m1,
                                               op0=ALU.mult, op1=ALU.mult)
                vv = rowp.tile([1, 512], f32, tag="row", name="vv")
                nc.vector.scalar_tensor_tensor(vv[:], qrow[0:1, :], 1.0 / D,
                                               msq[:], op0=ALU.mult,
                                               op1=ALU.subtract)
                return vv

            def rstd_dance(vv, dst3d):
                """vv [1,512] variance row -> 1/sqrt in [1,32,16] bf16 slice.

                Reshape to [128,4] via DMA so DVE's iterative reciprocal runs
                128 lanes wide, then Sqrt(1/v) on Scalar (kept adjacent with
                the other dances so the sqrt table loads once)."""
                dcol = rowp.tile([128, 4], f32, tag="dcol4", name="dcolv")
                nc.sync.dma_start(dcol[:], vv[:])
                rcp = rowp.tile([128, 4], f32, tag="rcp4", name="rcpv")
                nc.vector.reciprocal(rcp[:], dcol[:])
                rst = rowp.tile([128, 4], bf16, tag="rst4", name="rstv")
                nc.scalar.activation(rst[:], rcp[:], AF.Sqrt)
                nc.sync.dma_start(dst3d, rst[:])

            def p1_qkv(blk):
                tsl = slice(512 * blk, 512 * (blk + 1))
                xb = xbp.tile([128, DC, 512], fp8, tag="xb", name="xbq")
                nc.sync.dma_start(xb[:], xp[:, :, tsl])
                rps = psB.tile([128, 512], f32, tag="bc", name="rps")
                nc.tensor.matmul(rps[:], inv16_col[:],
                                 rstd1_all[0:1, 32 * blk:32 * (blk + 1), :],
                                 start=True, stop=True)
                rbc = gip.tile([128, 512], f32, tag="bcs", name="rbc")
                nc.scalar.activation(rbc[:], rps[:], AF.Copy)
                for n in range(3):
                    qps = psA.tile([128, 512], f32, tag="acc", name="qps")
                    for p in range(4):
                        nc.tensor.matmul(
                            qps[:], wqkv_sb[:, 2 * p:2 * p + 2,
                                            128 * n:128 * (n + 1)],
                            xb[:, 2 * p:2 * p + 2, :],
                            start=(p == 0), stop=False, perf_mode=DR)
                    nc.tensor.matmul(qps[:],
                                     ncsq_row[0:1, 128 * n:128 * (n + 1)],
                                     m1_all[0:1, tsl], start=False, stop=True)
                    nc.vector.tensor_tensor(qkvT[n][:, tsl], qps[:], rbc[:],
                                            op=ALU.mult)

            # =========== P2 attention pieces ===========
            def attn_vaug(b, h):
                hb = 64 * h
                bsl0 = S * b
                vaug = []
                for kp in range(8):           # kc pairs
                    va = vaugp.tile([128, 2, 128], fp8, tag="vaug", name="va")
                    nc.vector.memset(va[:, :, 64:65], 1.0)
                    nc.vector.memset(va[:, :, 65:128], 0.0)
                    for j in range(2):
                        kc = 2 * kp + j
                        tp = psT.tile([128, 64], bf16, tag="tp", name="tp")
                        nc.tensor.transpose(
                            tp[:], qkvT[2][hb:hb + 64,
                                           bsl0 + 128 * kc:bsl0 + 128 * (kc + 1)],
                            ident[hb:hb + 64, :])
                        nc.vector.tensor_copy(va[:, j, 0:64], tp[:])
                    vaug.append(va)
                return vaug

            def attn_qc(b, h, qc, vaug):
                hb = 64 * h
                bsl0 = S * b
                qsl = qkvT[0][hb:hb + 64, bsl0 + 512 * qc:bsl0 + 512 * (qc + 1)]
                exps = []
                for r in range(4):            # rounds of 4 kc
                    sc = psS.tile([128, 4, 512], f32, tag="sc", name="sc")
                    for j in range(4):
                        kc = 4 * r + j
                        nc.tensor.matmul(
                            sc[:, j, :],
                            qkvT[1][hb:hb + 64,
                                    bsl0 + 128 * kc:bsl0 + 128 * (kc + 1)],
                            qsl, start=True, stop=True)
                    e = expp.tile([128, 4, 512], fp8, tag="exp", name="e")
                    nc.scalar.activation(e[:], sc[:], AF.Exp)
                    exps.append(e)
                cps = psA.tile([128, 512], f32, tag="acc", name="cps")
                for r in range(4):
                    for j in range(2):
                        nc.tensor.matmul(cps[:], vaug[2 * r + j][:],
                                         exps[r][:, 2 * j:2 * j + 2, :],
                                         start=(r == 0 and j == 0),
                                         stop=(r == 3 and j == 1), perf_mode=DR)
                # denominator reciprocal via DVE (no ACT table involved)
                drow = rowp.tile([1, 128, 4], f32, tag="drow", name="drow")
                nc.vector.tensor_copy(drow[0:1, :, :], cps[64:65, :])
                dcol = rowp.tile([128, 4], f32, tag="dcol4", name="dcol4")
                nc.sync.dma_start(dcol[:], drow[0:1, :, :])
                dcr = rowp.tile([128, 4], bf16, tag="dcr4", name="dcr4")
                with nc.allow_low_precision(reason="bf16 softmax denom ok"):
                    nc.vector.reciprocal(dcr[:], dcol[:])
                rrow = rowp.tile([1, 128, 4], bf16, tag="rrow", name="rrow")
                nc.sync.dma_start(rrow[0:1, :, :], dcr[:])
                rb = psB.tile([128, 512], f32, tag="bc", name="rb")
                nc.tensor.matmul(rb[0:64, :], s32_row[:], rrow[0:1, :, :],
                                 start=True, stop=True)
                rbs = gip.tile([128, 512], f32, tag="bcs", name="rbs")
                nc.scalar.activation(rbs[0:64, :], rb[0:64, :], AF.Copy)
                cn = drp.tile([64, 512], bf16, tag="cn", name="cn")
                nc.vector.tensor_tensor(cn[:], cps[0:64, :], rbs[0:64, :],
                                        op=ALU.mult)
                nc.sync.dma_start(
                    ctxT[hb:hb + 64, bsl0 + 512 * qc:bsl0 + 512 * (qc + 1)],
                    cn[:])

            # =========== P3: attn-out partials ===========
            def p3(b):
                bsl0 = S * b
                for tcc in range(4):
                    for oc in range(DC):
                        ops3 = psA.tile([128, 512], f32, tag="acc", name="ops3")
                        nc.tensor.matmul(
                            ops3[:], ow_sb[:, 128 * oc:128 * (oc + 1)],
                            ctxT[:, bsl0 + 512 * tcc:bsl0 + 512 * (tcc + 1)],
                            start=True, stop=True)
                        po = drp.tile([128, 512], fp8, tag="po", name="po")
                        drain(po[:], ops3[:])
                        nc.sync.dma_start(
                            ar_in[b][128 * oc:128 * (oc + 1),
                                     512 * tcc:512 * (tcc + 1)], po[:])

            # =========== stripe stats -> AG payload ===========
            def stripe_stats(b):
                for tcc in range(4):
                    tsl = slice(512 * tcc, 512 * (tcc + 1))
                    gsl = slice(S * b + 512 * tcc, S * b + 512 * (tcc + 1))
                    with tc.high_priority(offset=-500000):
                        rs_t = wfp.tile([128, 512], fp8, tag="rs8", name="rs_t")
                        nc.sync.dma_start(rs_t[:], ag_in[b][0:128, tsl])
                    xot = wfp.tile([128, 512], f32, tag="xot", name="xot")
                    nc.sync.dma_start(xot[:], x_own[:, gsl])
                    rof = wfp.tile([128, 512], f32, tag="wf", name="rof")
                    nc.vector.scalar_tensor_tensor(rof[:], rs_t[:], 1.0 / SAR,
                                                   xot[:], op0=ALU.mult,
                                                   op1=ALU.add)
                    rof_bf = wfp.tile([128, 512], bf16, tag="wfb", name="rof_bf")
                    nc.vector.tensor_copy(rof_bf[:], rof[:])
                    sqf = wfp.tile([128, 512], bf16, tag="wfb", name="sqf")
                    nc.vector.tensor_tensor(sqf[:], rof[:], rof[:], op=ALU.mult)
                    srow = psA.tile([128, 512], f32, tag="acc", name="ssrow")
                    nc.tensor.matmul(srow[0:1, :], ones_col[:], rof_bf[:],
                                     start=True, stop=True)
                    qrow = psA.tile([128, 512], f32, tag="acc", name="sqrow")
                    nc.tensor.matmul(qrow[0:1, :], ones_col[:], sqf[:],
                                     start=True, stop=True)
                    st8a = rowp.tile([1, 512], fp8, tag="st8", name="st8a")
                    nc.vector.tensor_scalar(st8a[:], srow[0:1, :],
                                            1.0 / SAR, None, op0=ALU.mult)
                    st8b = rowp.tile([1, 512], fp8, tag="st8", name="st8b")
                    nc.vector.tensor_scalar(st8b[:], qrow[0:1, :],
                                            1.0 / SAR, None, op0=ALU.mult)
                    nc.sync.dma_start(ag_in[b][128:129, tsl], st8a[:])
                    nc.sync.dma_start(ag_in[b][129:130, tsl], st8b[:])

            # =========== P4 stats (batched per b, after AG) ===========
            def p4_stats(b):
                st = sqp.tile([16, S], fp8, tag="st", name="st", bufs=1)
                with tc.high_priority(offset=-500000):
                    nc.sync.dma_start(st[:], ag_out[b][0:NC, 128:130, :])
                for tcc in range(4):
                    tsl = slice(512 * tcc, 512 * (tcc + 1))
                    tot = psA.tile([128, 512], f32, tag="acc", name="tot")
                    nc.tensor.matmul(tot[0:1, :], sel[:, 0:1], st[:, tsl],
                                     start=True, stop=True)
                    totq = psA.tile([128, 512], f32, tag="acc", name="totq")
                    nc.tensor.matmul(totq[0:1, :], sel[:, 1:2], st[:, tsl],
                                     start=True, stop=True)
                    m2 = m2_all[b][0:1, tsl]
                    nc.vector.tensor_copy(m2, tot[0:1, :])
                    msq = rowp.tile([1, 512], f32, tag="row", name="msq2")
                    nc.vector.scalar_tensor_tensor(
                        msq[:], m2, 1.0 / (SW * SW), m2,
                        op0=ALU.mult, op1=ALU.mult)
                    vv = rowp.tile([1, 512], f32, tag="row", name="vv2")
                    nc.vector.scalar_tensor_tensor(
                        vv[:], totq[0:1, :], 1.0 / SW, msq[:],
                        op0=ALU.mult, op1=ALU.subtract)
                    rstd_dance(vv, rstd2_all[b][0:1, 32 * tcc:32 * (tcc + 1), :])

            # =========== P4: MLP block (one 512-token chunk) ===========
            def p4(b, tcc):
                tsl = slice(512 * tcc, 512 * (tcc + 1))
                gsl = slice(S * b + 512 * tcc, S * b + 512 * (tcc + 1))
                ag_t, rs_t = [], []
                for p in range(4):
                    ag = agp.tile([128, 2, 512], fp8, tag="ag", name="ag")
                    with tc.high_priority(offset=-500000):
                        for j in range(2):
                            d = 2 * p + j
                            nc.sync.dma_start(ag[:, j, :],
                                              ag_out[b][d:d + 1, 0:128, tsl])
                    xpr = rsdp.tile([128, 2, 512], fp8, tag="xpr", name="xpr")
                    nc.sync.dma_start(xpr[:], xp[:, 2 * p:2 * p + 2, gsl])
                    rsd = rsdp.tile([128, 2, 512], fp8, tag="rsd", name="rsd")
                    nc.vector.scalar_tensor_tensor(
                        rsd[:], ag[:], 1.0 / SAR, xpr[:],
                        op0=ALU.mult, op1=ALU.add)
                    ag_t.append(ag)
                    rs_t.append(rsd)

                rps = psB.tile([128, 512], f32, tag="bc", name="rps2")
                nc.tensor.matmul(rps[:], inv16_col[:],
                                 rstd2_all[b][0:1, 32 * tcc:32 * (tcc + 1), :],
                                 start=True, stop=True)
                rbc = gip.tile([128, 512], f32, tag="bcs", name="rbc2")
                nc.scalar.activation(rbc[:], rps[:], AF.Copy)

                it_t = [itp.tile([128, 2, 512], fp8, tag="it", name="it")
                        for _ in range(2)]
                for ic in range(4):
                    h2ps = psA.tile([128, 512], f32, tag="acc", name="h2ps")
                    for p in range(4):
                        nc.tensor.matmul(
                            h2ps[:], w2_sb[:, 2 * p:2 * p + 2,
                                           128 * ic:128 * (ic + 1)],
                            ag_t[p][:], start=(p == 0), stop=(p == 3),
                            perf_mode=DR)
                    h1ps = psA.tile([128, 512], f32, tag="acc", name="h1ps")
                    for p in range(4):
                        nc.tensor.matmul(
                            h1ps[:], w1_sb[:, 2 * p:2 * p + 2,
                                           128 * ic:128 * (ic + 1)],
                            rs_t[p][:], start=(p == 0), stop=False,
                            perf_mode=DR)
                    nc.tensor.matmul(h1ps[:],
                                     ncs1_row[0:1, 128 * ic:128 * (ic + 1)],
                                     m2_all[b][0:1, tsl], start=False, stop=True)
                    gi = gip.tile([128, 512], f32, tag="gi", name="gi")
                    nc.vector.tensor_tensor(gi[:], h1ps[:], rbc[:], op=ALU.mult)
                    h1 = h1p.tile([128, 512], bf16, tag="h1", name="h1")
                    nc.scalar.activation(h1[:], gi[:],
                                         AF.Sigmoid if SIM_GELU else AF.Gelu)
                    nc.vector.scalar_tensor_tensor(
                        it_t[ic // 2][:, ic % 2, :], h2ps[:], 1.0 / SW, h1[:],
                        op0=ALU.mult, op1=ALU.mult)
                for oc in range(DC):
                    ops3 = psA.tile([128, 512], f32, tag="acc", name="ops4")
                    for icp in range(2):
                        nc.tensor.matmul(
                            ops3[:], outw_sb[:, 2 * icp:2 * icp + 2,
                                             128 * oc:128 * (oc + 1)],
                            it_t[icp][:], start=(icp == 0), stop=(icp == 1),
                            perf_mode=DR)
                    po2 = drp.tile([128, 512], fp8, tag="po", name="po2")
                    drain(po2[:], ops3[:])
                    nc.sync.dma_start(
                        rs2_in[2 * b + tcc // 2][128 * oc:128 * (oc + 1),
                                                 512 * (tcc % 2):
                                                 512 * (tcc % 2 + 1)], po2[:])

            # =========== P6: final stripe chunks ===========
            def p6(b, tcc):
                tsl = slice(512 * tcc, 512 * (tcc + 1))
                gsl = slice(S * b + 512 * tcc, S * b + 512 * (tcc + 1))
                with tc.high_priority(offset=-500000):
                    rs_t = wfp.tile([128, 512], fp8, tag="rs8", name="rs_t6")
                    nc.sync.dma_start(rs_t[:], ag_in[b][0:128, tsl])
                    r2 = wfp.tile([128, 512], fp8, tag="rs8", name="r2")
                    nc.sync.dma_start(
                        r2[:], rs2_out[2 * b + tcc // 2][:, 512 * (tcc % 2):
                                                         512 * (tcc % 2 + 1)])
                xot = wfp.tile([128, 512], f32, tag="xot", name="xot6")
                nc.sync.dma_start(xot[:], x_own[:, gsl])
                t1 = wfp.tile([128, 512], f32, tag="wf", name="t1")
                nc.vector.scalar_tensor_tensor(t1[:], rs_t[:], 1.0 / SAR,
                                               xot[:], op0=ALU.mult, op1=ALU.add)
                ot = wfp.tile([128, 512], f32, tag="wf", name="ot")
                nc.vector.scalar_tensor_tensor(ot[:], r2[:], 1.0 / SO2, t1[:],
                                               op0=ALU.mult, op1=ALU.add)
                nc.sync.dma_start(outT[:, gsl], ot[:])

            # ================= EMISSION =================
            # LN1 stats; p1_stats touches no ACT, so the per-block Sqrt
            # dances stay on one activation-table set
            for blk in range(8):
                vv = p1_stats(blk)
                rstd_dance(vv, rstd1_all[0:1, 32 * blk:32 * (blk + 1), :])
            # QKV for batch-0 tokens
            for blk in range(4):
                p1_qkv(blk)

            # MLP weights now (DMA overlaps attention)
            w1_sb = wp.tile([128, DC, 512], fp8, name="w1_sb")
            nc.sync.dma_start(w1_sb[:], w1[:])
            w2_sb = wp.tile([128, DC, 512], fp8, name="w2_sb")
            nc.sync.dma_start(w2_sb[:], w2[:])
            outw_sb = wp.tile([128, 4, D], fp8, name="outw_sb")
            nc.sync.dma_start(outw_sb[:], outw[:])

            # attention(b0) interleaved with QKV blocks 4..7 (batch-1 tokens)
            rem = [4, 5, 6, 7]
            for h in range(2):
                vaug = attn_vaug(0, h)
                for qc in range(4):
                    if rem:
                        p1_qkv(rem.pop(0))
                    attn_qc(0, h, qc, vaug)

            p3(0)
            nc.gpsimd.collective_compute(
                "ReduceScatter", ALU.add, ins=[ar_in[0].opt()],
                outs=[ag_in[0][0:128, :].opt()], replica_groups=RG)

            # attention(b1) fully covers RS(b0) latency
            for h in range(2):
                vaug = attn_vaug(1, h)
                for qc in range(4):
                    attn_qc(1, h, qc, vaug)

            stripe_stats(0)
            nc.gpsimd.collective_compute(
                "AllGather", ALU.bypass, ins=[ag_in[0].opt()],
                outs=[ag_out[0].opt()], replica_groups=RG)

            p3(1)
            nc.gpsimd.collective_compute(
                "ReduceScatter", ALU.add, ins=[ar_in[1].opt()],
                outs=[ag_in[1][0:128, :].opt()], replica_groups=RG)

            p4_stats(0)
            p4(0, 0)
            p4(0, 1)
            stripe_stats(1)
            nc.gpsimd.collective_compute(
                "AllGather", ALU.bypass, ins=[ag_in[1].opt()],
                outs=[ag_out[1].opt()], replica_groups=RG)
            nc.gpsimd.collective_compute(
                "ReduceScatter", ALU.add, ins=[rs2_in[0].opt()],
                outs=[rs2_out[0].opt()], replica_groups=RG)
            p4(0, 2)
            p4(0, 3)
            p4_stats(1)
            nc.gpsimd.collective_compute(
                "ReduceScatter", ALU.add, ins=[rs2_in[1].opt()],
                outs=[rs2_out[1].opt()], replica_groups=RG)
            p4(1, 0)
            p4(1, 1)
            nc.gpsimd.collective_compute(
                "ReduceScatter", ALU.add, ins=[rs2_in[2].opt()],
                outs=[rs2_out[2].opt()], replica_groups=RG)
            p6(0, 0)
            p6(0, 1)
            p4(1, 2)
            p4(1, 3)
            nc.gpsimd.collective_compute(
                "ReduceScatter", ALU.add, ins=[rs2_in[3].opt()],
                outs=[rs2_out[3].opt()], replica_groups=RG)
            p6(0, 2)
            p6(0, 3)
            for tcc in range(4):
                p6(1, tcc)

    nc.compile()
    return nc


_NC_CACHE = {}


def make_in_maps(**inputs):
    x = np.asarray(inputs["x"], np.float32)
    norm_w = np.asarray(inputs["norm_w"], np.float32)
    norm_b = np.asarray(inputs["norm_b"], np.float32)
    qkvw = np.asarray(inputs["attn_qkvw"], np.float32)
    qkvb = np.asarray(inputs["attn_qkvb"], np.float32)
    attn_ow = np.asarray(inputs["attn_ow"], np.float32)
    attn_ob = np.asarray(inputs["attn_ob"], np.float32)
    attn_nw = np.asarray(inputs["attn_nw"], np.float32)
    attn_nb = np.asarray(inputs["attn_nb"], np.float32)
    inter_w = np.asarray(inputs["inter_w"], np.float32)
    inter_b = np.asarray(inputs["inter_b"], np.float32)
    inter_w1 = np.asarray(inputs["inter_w1"], np.float32)
    output_w = np.asarray(inputs["output_w"], np.float32)
    output_b = np.asarray(inputs["output_b"], np.float32)

    X = x.reshape(T, D)
    XT = np.ascontiguousarray(X.T)              # [D, T]

    # LN folds
    wqkv_f = norm_w[:, None] * qkvw
    bqkv_f = qkvb + norm_b @ qkvw
    wqkv_f = wqkv_f.copy()
    wqkv_f[:, :D] /= np.sqrt(HD)
    w1_f = attn_nw[:, None] * inter_w
    b1_f = inter_b + attn_nb @ inter_w

    assert not np.any(bqkv_f) and not np.any(attn_ob) and not np.any(b1_f) \
        and not np.any(output_b), "nonzero biases not wired in this build"

    xp_all = _f8(_planes(XT))                   # x planes [128, 8, T]

    in_maps = []
    for c in range(NC):
        hsl = slice(128 * c, 128 * (c + 1))
        isl = slice(512 * c, 512 * (c + 1))
        wq_c = np.concatenate(
            [wqkv_f[:, hsl], wqkv_f[:, D:][:, hsl], wqkv_f[:, 2 * D:][:, hsl]],
            axis=1)                             # [D, 384]
        wq8 = _f8(_planes(wq_c * SW))
        w1_c = w1_f[:, isl]
        w18 = _f8(_planes(w1_c * SW))
        w28 = _f8(_planes(inter_w1[:, isl] * (SCTX * SW / SAR)))
        ou8 = _f8(output_w[isl, :].reshape(4, 128, D).transpose(1, 0, 2)
                  * (SO2 / SCTX))
        wq_deq = wq8.astype(np.float32).transpose(1, 0, 2).reshape(D, NQKV)
        w1_deq = w18.astype(np.float32).transpose(1, 0, 2).reshape(D, 512)
        in_maps.append({
            "xp": xp_all,
            "x_own": np.ascontiguousarray(XT[hsl, :]),
            "wqkv": wq8,
            "ncsq": np.ascontiguousarray(-wq_deq.sum(0, keepdims=True) / D),
            "ow": _bf(attn_ow[hsl, :] * (SAR / SCTX)),
            "w1": w18,
            "ncs1": np.ascontiguousarray(
                -w1_deq.sum(0, keepdims=True) / SW),
            "w2": w28,
            "outw": np.ascontiguousarray(ou8),
            "seli": _f8(np.tile(np.eye(2, dtype=np.float32), (8, 1))),
        })
    return in_maps


def kernel(**inputs):
    if "nc" not in _NC_CACHE:
        _NC_CACHE["nc"] = _build()
    nc = _NC_CACHE["nc"]
    in_maps = make_in_maps(**inputs)
    global _LAST_IN_MAPS
    _LAST_IN_MAPS = in_maps
    res = run_bass_kernel_spmd(nc, in_maps, list(range(NC)))
    outT = np.concatenate([res.results[c]["outT"] for c in range(NC)], axis=0)
    return np.ascontiguousarray(outT.T).reshape(B, S, D).astype(np.float32)


if __name__ == "__main__":
    pass
